# revision 11
# baseline (speedup 1.0000x reference)
"""KGATConv GNN message-passing kernel for 8 Trainium2 NeuronCores.

Strategy (dst-node ownership, no collectives):
  - Core k owns nodes [k*12500, (k+1)*12500).
  - Host sorts edges by dst and buckets per (core, 128-node window), padding
    each window's edge run to whole 128-edge chunks (chunk counts shared
    across cores so all 8 run one SPMD program).
  - Device, per chunk: indirect-DMA gather of 128 nfeat[src] rows (one offset
    per partition -- the only indirect mode this toolchain executes
    correctly); DVE builds A[p,j] = w_p * (dst_p == j); PE matmul-accumulates
    h_nb = A^T @ msg in PSUM.  Finalize per window: X = nfeat_own * h_nb,
    X^T via PE transpose, out = X @ W^T on PE, LeakyReLU on ACT, DMA out.
"""

import sys

sys.path.insert(0, "/opt/trn_rl_repo")

from contextlib import ExitStack

import numpy as np

import concourse.bass as bass
import concourse.mybir as mybir
import concourse.tile as tile
from concourse.bass_utils import run_bass_kernel_spmd

N_CORES = 8
D = 128
WIN = 128

_nc_cache = {}


def _split_excess_waits(nc, maxw=1):
    # This walrus build rejects instructions carrying more than one sync
    # wait; move extras onto preceding single-wait NoOps on the same engine.
    for f in nc.m.functions:
        for bb in f.blocks:
            out = []
            for inst in bb.instructions:
                si = inst.sync_info
                waits = list(si.on_wait) if si and si.on_wait else []
                if len(waits) > maxw:
                    extra, keep = waits[:-maxw], waits[-maxw:]
                    for i in range(0, len(extra), maxw):
                        nop = mybir.InstNoOp(
                            name=nc.get_next_instruction_name(), ins=[], outs=[]
                        )
                        nop.engine = inst.engine
                        nop.sync_info = type(si)(
                            on_wait=extra[i : i + maxw], on_update=[]
                        )
                        nc.register_instruction(nop, overwrite=True)
                        out.append(nop)
                    si.on_wait = keep
                out.append(inst)
            bb.instructions[:] = out


def _build_nc(n_rows, nw, ct, c_list):
    f32 = mybir.dt.float32
    nc = bass.Bass()
    nfeat_d = nc.declare_dram_parameter("nfeat", [n_rows, D], f32, isOutput=False)
    nfown_d = nc.declare_dram_parameter("nfown", [nw * WIN, D], f32, isOutput=False)
    src_d = nc.declare_dram_parameter("src", [128, ct], mybir.dt.int32, isOutput=False)
    dst_d = nc.declare_dram_parameter("dstf", [128, ct], f32, isOutput=False)
    w_d = nc.declare_dram_parameter("wf", [128, ct], f32, isOutput=False)
    wt_d = nc.declare_dram_parameter("wt", [D, D], f32, isOutput=False)
    iota_d = nc.declare_dram_parameter("iota", [128, WIN], f32, isOutput=False)
    ident_d = nc.declare_dram_parameter("ident", [128, 128], f32, isOutput=False)
    out_d = nc.declare_dram_parameter("out", [nw * WIN, D], f32, isOutput=True)

    with tile.TileContext(nc) as tc, ExitStack() as ctx:
        const = ctx.enter_context(tc.tile_pool(name="const", bufs=1))
        gp = ctx.enter_context(tc.tile_pool(name="gp", bufs=10))
        ap = ctx.enter_context(tc.tile_pool(name="ap", bufs=4))
        wk = ctx.enter_context(tc.tile_pool(name="wk", bufs=3))
        ps = ctx.enter_context(tc.tile_pool(name="ps", bufs=2, space="PSUM"))

        src_sb = const.tile([128, ct], mybir.dt.int32)
        nc.sync.dma_start(out=src_sb[:], in_=src_d[:])
        dst_sb = const.tile([128, ct], f32)
        nc.sync.dma_start(out=dst_sb[:], in_=dst_d[:])
        w_sb = const.tile([128, ct], f32)
        nc.sync.dma_start(out=w_sb[:], in_=w_d[:])
        wt_sb = const.tile([D, D], f32)
        nc.sync.dma_start(out=wt_sb[:], in_=wt_d[:])
        iota_sb = const.tile([128, WIN], f32)
        nc.sync.dma_start(out=iota_sb[:], in_=iota_d[:])
        ident_sb = const.tile([128, 128], f32)
        nc.sync.dma_start(out=ident_sb[:], in_=ident_d[:])

        start = 0
        for t in range(nw):
            c = c_list[t]
            acc = ps.tile([WIN, D], f32, tag="acc")
            for j in range(c):
                col = start + j
                # one offset per partition; dest [128,128] = one nfeat row
                # per partition (the only indirect mode this walrus build
                # executes correctly).
                g = gp.tile([128, D], f32, tag="g")
                nc.gpsimd.indirect_dma_start(
                    out=g[:],
                    out_offset=None,
                    in_=nfeat_d[:],
                    in_offset=bass.IndirectOffsetOnAxis(
                        ap=src_sb[:, col : col + 1], axis=0
                    ),
                )
                a_t = ap.tile([128, WIN], f32, tag="A")
                nc.vector.tensor_scalar(
                    a_t[:],
                    iota_sb[:],
                    dst_sb[:, col : col + 1],
                    w_sb[:, col : col + 1],
                    mybir.AluOpType.is_equal,
                    mybir.AluOpType.mult,
                )
                nc.tensor.matmul(
                    out=acc[:],
                    lhsT=a_t[:],
                    rhs=g[:],
                    start=(j == 0),
                    stop=(j == c - 1),
                )
            nf = wk.tile([WIN, D], f32, tag="nf")
            nc.sync.dma_start(out=nf[:], in_=nfown_d[t * WIN : (t + 1) * WIN, :])
            x = wk.tile([WIN, D], f32, tag="x")
            nc.vector.tensor_tensor(
                out=x[:], in0=nf[:], in1=acc[:], op=mybir.AluOpType.mult
            )
            xt_ps = ps.tile([D, WIN], f32, tag="xt")
            nc.tensor.transpose(out=xt_ps[:], in_=x[:], identity=ident_sb[:])
            xt = wk.tile([D, WIN], f32, tag="xts")
            nc.scalar.activation(
                out=xt[:], in_=xt_ps[:], func=mybir.ActivationFunctionType.Copy
            )
            op_ps = ps.tile([WIN, D], f32, tag="op")
            nc.tensor.matmul(
                out=op_ps[:], lhsT=xt[:], rhs=wt_sb[:], start=True, stop=True
            )
            ob = wk.tile([WIN, D], f32, tag="ob")
            nc.scalar.activation(
                out=ob[:],
                in_=op_ps[:],
                func=mybir.ActivationFunctionType.Lrelu,
                alpha=0.01,
            )
            nc.sync.dma_start(out=out_d[t * WIN : (t + 1) * WIN, :], in_=ob[:])
            start += c
    _split_excess_waits(nc)
    return nc


def _kernel_impl(nfeat, edge_src, edge_dst, edge_w, W, npc, trace=False):
    n, d = nfeat.shape
    assert d == D and npc * N_CORES == n
    nw = (npc + WIN - 1) // WIN

    order = np.argsort(edge_dst, kind="stable")
    ds = edge_dst[order].astype(np.int64)
    ss = edge_src[order].astype(np.int64)
    ws = edge_w[order].astype(np.float32)

    bounds = []
    for k in range(N_CORES):
        base = k * npc
        for t in range(nw):
            bounds.append(min(base + t * WIN, base + npc))
    bounds.append(N_CORES * npc)
    idx = np.searchsorted(ds, np.array(bounds))
    cnts = np.diff(idx).reshape(N_CORES, nw)
    pos = idx[:-1].reshape(N_CORES, nw)

    c_list = [int(max(1, v)) for v in np.ceil(cnts / 128).max(axis=0).astype(int)]
    ct = int(sum(c_list))
    starts = np.concatenate([[0], np.cumsum(c_list)[:-1]]).astype(int)

    src_arr = np.zeros((N_CORES, 128, ct), np.int32)
    dst_arr = np.zeros((N_CORES, 128, ct), np.float32)
    w_arr = np.zeros((N_CORES, 128, ct), np.float32)
    for k in range(N_CORES):
        for t in range(nw):
            cnt = int(cnts[k, t])
            if cnt == 0:
                continue
            o0 = int(pos[k, t])
            j = np.arange(cnt)
            col = starts[t] + (j // 128)
            row = j % 128
            src_arr[k, row, col] = ss[o0 : o0 + cnt]
            dst_arr[k, row, col] = (ds[o0 : o0 + cnt] - (k * npc + t * WIN)).astype(
                np.float32
            )
            w_arr[k, row, col] = ws[o0 : o0 + cnt]

    wt = np.ascontiguousarray(W.T.astype(np.float32))
    iota = np.tile(np.arange(WIN, dtype=np.float32), (128, 1))
    ident = np.eye(128, dtype=np.float32)
    nfeat = np.ascontiguousarray(nfeat.astype(np.float32))

    key = (n, npc, ct, tuple(c_list))
    if key not in _nc_cache:
        _nc_cache[key] = _build_nc(n, nw, ct, c_list)
    nc = _nc_cache[key]

    in_maps = []
    for k in range(N_CORES):
        nfown = np.zeros((nw * WIN, D), np.float32)
        lo = k * npc
        avail = min(nw * WIN, n - lo)
        nfown[:avail] = nfeat[lo : lo + avail]
        in_maps.append(
            {
                "nfeat": nfeat,
                "nfown": nfown,
                "src": src_arr[k],
                "dstf": dst_arr[k],
                "wf": w_arr[k],
                "wt": wt,
                "iota": iota,
                "ident": ident,
            }
        )

    r = run_bass_kernel_spmd(nc, in_maps, list(range(N_CORES)), trace=trace)
    out = np.empty((n, D), np.float32)
    for k in range(N_CORES):
        out[k * npc : (k + 1) * npc] = r.results[k]["out"][:npc]
    if trace:
        return out, r
    return out


def kernel(nfeat, edge_src, edge_dst, edge_w, W):
    return _kernel_impl(
        np.asarray(nfeat),
        np.asarray(edge_src),
        np.asarray(edge_dst),
        np.asarray(edge_w),
        np.asarray(W),
        npc=12500,
    )
